# revision 3
# baseline (speedup 1.0000x reference)
"""Swin-style window attention kernel for 8 TRN2 NeuronCores (SPMD, batch-sharded).

Layout strategy per core (16 windows, processed in pairs):
  - host prep: x pre-transposed to [pair, ki, ko, tok] in fp8e4m3 plus an
    fp8 residual copy xr8 = fp8(x - fp8(x)); q,k,v weights x64-scaled to
    fp8 (wv also gets an fp8 residual wvr); proj weights bf16; exp(rel-pos
    bias) precomputed as a bf16 table.
  - q,k projection: fp8 DoubleRow matmuls (2 k-tiles/instr, 0.5 cyc/row),
    evacuated fp8 into persistent ping-pong tiles whose second slot is
    all-zero - the zero slot is the mate k-tile that lets the 32-deep
    QK^T contraction also run in DoubleRow mode at 0.5 cyc/row.
  - v projection token-major [tok,(h,d)] via 3-term fp8 DoubleRow
    (x8@wv + x8@wvr + xr8@wv, one shared x64-scaled psum): ~bf16 accuracy
    at fp8-DR cost. x8/xr8 sbuf tiles are 400-padded (DR Ldweights needs
    Ko stride % 16 == 0).
  - attention in 2-head sub-groups, double-buffered QK psum: QK^T
    transposed (attnT [k,196]) fp8-DR 32-row-packed matmuls; exp on ACT
    with the softmax 1/sqrt(d) as activation pre-scale; *exp(bias) on DVE
    (bf16 2x); AV dense + replicated denominators via ones-matmuls into a
    4-head psum bank; reciprocal + normalize on DVE into per-group
    attn_r tiles (so proj block g depends only on group g).
  - pipeline: w0's first QK sub-groups issued mid-qkv; QK(sg+2) staggered
    ahead of AV(sg); w0 proj inline after w1's first QK; w1 proj deferred
    into the next pair's qkv stage on the idle attention-psum pool. Last
    pair: w1's proj interleaved per-group into the attention stream.
  - proj bf16, bias added during psum evacuation (DVE); y stores + x
    prefetch on the SP queue, weights on ACT hwdge, exp(bias) table in
    4-head chunks on gpsimd swdge.
"""

import numpy as np

B, NT, CH = 128, 196, 512
H, D = 16, 32
NCORES = 8
WPC = B // NCORES  # windows per core

_CACHE = {}


def _build():
    import concourse.bass as bass
    import concourse.mybir as mybir
    import concourse.tile as tile
    from concourse import bacc

    fp32 = mybir.dt.float32
    bf16 = mybir.dt.bfloat16
    fp16 = mybir.dt.float16
    f8 = mybir.dt.float8e4
    DR = mybir.MatmulPerfMode.DoubleRow
    AF = mybir.ActivationFunctionType

    nc = bacc.Bacc("TRN2", target_bir_lowering=False, debug=False, num_devices=NCORES)

    # x pre-transposed on host: [pair, ki, ko, wi*196+q] = x[w, q, 128*ko+ki]
    # fp8 copy feeds q,k AND v (v precision recovered via the fp8 residual
    # copy xr8 and residual weights wvr8: x@wv ~ x8@wv8 + x8@wvr8 + xr8@wv8,
    # all three terms x64-scaled so they share one psum).
    x8d = nc.dram_tensor("x8", [WPC // 2, 128, 4, 2 * NT], f8, kind="ExternalInput")
    xr8d = nc.dram_tensor("xr8", [WPC // 2, 128, 4, 2 * NT], f8, kind="ExternalInput")
    wqk_d = nc.dram_tensor("wqk", [CH, 2 * CH], f8, kind="ExternalInput")
    wv_d = nc.dram_tensor("wv", [CH, CH], f8, kind="ExternalInput")
    wvr_d = nc.dram_tensor("wvr", [CH, CH], f8, kind="ExternalInput")
    ebt_d = nc.dram_tensor("ebt", [128, H, 2 * NT], bf16, kind="ExternalInput")
    proj_w = nc.dram_tensor("proj_w", [CH, CH], bf16, kind="ExternalInput")
    proj_b = nc.dram_tensor("proj_b", [1, CH], fp32, kind="ExternalInput")
    pb16_d = nc.dram_tensor("pb16", [1, CH], bf16, kind="ExternalInput")
    y = nc.dram_tensor("y", [WPC, NT, CH], fp32, kind="ExternalOutput")

    with tile.TileContext(nc) as tc:
        with (
            tc.tile_pool(name="const", bufs=1) as cpool,
            tc.tile_pool(name="work", bufs=2) as wpool,
            tc.tile_pool(name="attn", bufs=6) as apool,
            tc.tile_pool(name="ps_qk", bufs=2, space="PSUM") as ps_qk,
            tc.tile_pool(name="ps_av", bufs=2, space="PSUM") as ps_av,
            tc.tile_pool(name="ps_ms", bufs=2, space="PSUM") as ps_ms,
        ):
            # ---------------- one-time setup ----------------
            # persistent fp8 q,k feature tiles (ping-pong across pairs).
            # Slot a=1 stays all-zero: it is the second k-tile of every
            # DoubleRow QK^T matmul (the PE streams 2 tiles/cycle in DR mode,
            # so a 32-deep contraction runs at 0.5 cyc/row with a zero mate).
            # First tile zeroed on idle DVE right away (needed ~4us in);
            # second on Pool (not needed until pair 1).
            qk8s = []
            for pp_ in range(2):
                qk8t = cpool.tile([128, 2, 8, 2 * NT], f8, tag=f"qk8_{pp_}",
                                  name=f"qk8_{pp_}")
                nc.gpsimd.memset(qk8t[:, 1, :, :], 0.0)
                qk8s.append(qk8t)

            # x loads own the SP queue; weights go via ACT hwdge (in
            # first-needed order); small/late tensors via gpsimd swdge.
            def load_x(pair):
                # x8 split at the DoubleRow kt boundary: the first qkv
                # matmul only needs chunks 0-1, so it can start ~1us sooner.
                # xr8 (residual) is only needed at the v stage, so it loads
                # after both x8 chunks.
                # inner padded 392->400: DoubleRow Ldweights needs the Ko
                # slot stride to be a multiple of 16
                x8 = wpool.tile([128, 4, 400], f8, tag=f"x8{pair % 2}")
                xr8 = wpool.tile([128, 4, 400], f8, tag=f"xr8{pair % 2}")
                nc.sync.dma_start(x8[:, 0:2, 0:2 * NT], x8d.ap()[pair, :, 0:2, :])
                nc.sync.dma_start(x8[:, 2:4, 0:2 * NT], x8d.ap()[pair, :, 2:4, :])
                nc.sync.dma_start(xr8[:, :, 0:2 * NT], xr8d.ap()[pair])
                return xr8, x8

            xs_cur = load_x(0)

            wqk = cpool.tile([128, 4, 2 * CH], f8, tag="wqk")
            wqk_r = wqk_d.ap().rearrange("(ko ki) m -> ki ko m", ki=128)
            nc.scalar.dma_start(wqk[:, 0:2, :], wqk_r[:, 0:2, :])
            nc.scalar.dma_start(wqk[:, 2:4, :], wqk_r[:, 2:4, :])
            wv = cpool.tile([128, 4, CH], f8, tag="wv")
            nc.scalar.dma_start(wv[:], wv_d.ap().rearrange("(ko ki) m -> ki ko m", ki=128))
            wvr = cpool.tile([128, 4, CH], f8, tag="wvr")
            nc.scalar.dma_start(wvr[:], wvr_d.ap().rearrange("(ko ki) m -> ki ko m", ki=128))
            # ebt in 4-head chunks so the big table transfer never delays the
            # startup-critical x/weight DMAs; chunk g unblocks groups in order
            ebt = cpool.tile([128, H, 2 * NT], bf16, tag="ebt")
            for g4 in range(4):
                nc.gpsimd.dma_start(ebt[:, 4 * g4:4 * g4 + 4, :],
                                    ebt_d.ap()[:, 4 * g4:4 * g4 + 4, :])
            pw = cpool.tile([128, 4, CH], bf16, tag="pw")
            nc.scalar.dma_start(pw[:], proj_w.ap().rearrange("(ko ki) m -> ki ko m", ki=128))

            b_row = cpool.tile([1, CH], fp32, tag="brow")
            nc.gpsimd.dma_start(b_row[:], proj_b.ap())
            b_bcast = cpool.tile([128, CH], fp32, tag="bb")
            nc.gpsimd.partition_broadcast(b_bcast[:], b_row[:], channels=128)

            ones32 = cpool.tile([128, 32], bf16, tag="ones")
            nc.gpsimd.memset(ones32[:], 1.0)
            # tail-only: bias row (bf16) + ones row for the last pair's
            # rank-1 bias matmul
            b16 = cpool.tile([1, CH], bf16, tag="b16")
            nc.gpsimd.dma_start(b16[:], pb16_d.ap())
            ones_row = cpool.tile([1, 128], bf16, tag="onesr")
            nc.gpsimd.memset(ones_row[:], 1.0)

            pending = []  # deferred proj emission (fills next pair's qkv stage)

            def emit_qk(qk8, wi, sg):
                wo = wi * NT
                qkps = ps_qk.tile([128, 2, 512], fp32, tag="qkps")
                for j in range(2):
                    h = 2 * sg + j
                    hb = 32 * (h % 4)
                    qblk, kblk = h // 4, 4 + h // 4
                    rhs_q = qk8[hb:hb + 32, :, qblk, wo:wo + NT]
                    nc.tensor.matmul(qkps[:, j, 0:NT],
                                     qk8[hb:hb + 32, :, kblk, wo:wo + 128],
                                     rhs_q, start=True, stop=True,
                                     tile_position=(hb, 0), perf_mode=DR)
                    if wi == 0:
                        nc.tensor.matmul(qkps[:, j, NT:2 * NT],
                                         qk8[hb:hb + 32, :, kblk, 128:256],
                                         rhs_q, start=True, stop=True,
                                         tile_position=(hb, 0), perf_mode=DR)
                    else:
                        nc.tensor.matmul(qkps[0:68, j, NT:2 * NT],
                                         qk8[hb:hb + 32, :, kblk, wo + 128:wo + NT],
                                         rhs_q, start=True, stop=True,
                                         tile_position=(hb, 0), perf_mode=DR)
                return qkps

            def attn_window(qk8, v_sb, wi, qlist, proj_hook=None):
                # qlist: already-emitted qkps tiles (pipeline pre-fill)
                P = len(qlist)
                # one tile per 4-head group so proj block bl only depends on
                # its own group's normalize (not the last one)
                attn_rs = [apool.tile([128, NT], bf16, tag=f"attn_r{g}",
                                      name=f"attn_r{g}")
                           for g in range(4)]
                avps = None
                for sg in range(8):
                    qkps = qlist[sg]
                    # exp (ACT) then *exp(bias); the softmax 1/sqrt(d) rides
                    # along as the activation pre-scale. The bias multiply
                    # alternates DVE/Pool so neither elementwise engine
                    # serializes the attention stream.
                    esb = apool.tile([128, 2, 2 * NT], bf16, tag="esb")
                    nc.scalar.activation(esb[:], qkps[:, :, 0:2 * NT], AF.Exp,
                                         scale=0.17677669529663687)
                    et = apool.tile([128, 2, 2 * NT], bf16, tag="et")
                    nc.vector.tensor_mul(et[:], esb[:], ebt[:, 2 * sg:2 * sg + 2, :])
                    if sg + P < 8:
                        qlist.append(emit_qk(qk8, wi, sg + P))
                    # AV + replicated denominators into 4-head psum bank
                    if sg % 2 == 0:
                        avps = ps_av.tile([128, 512], fp32, tag="avps")
                    # ones (denominator) matmuls FIRST: the reciprocal can
                    # then overlap the AV matmuls instead of serializing
                    # after them on the window-end critical path
                    for j in range(2):
                        h = 2 * sg + j
                        band = 32 * (h % 4)
                        nc.tensor.matmul(avps[band:band + 32, 256:256 + NT],
                                         ones32[:], et[:, j, 0:NT],
                                         start=True, stop=False,
                                         tile_position=(0, band))
                        nc.tensor.matmul(avps[band:band + 32, 256:256 + NT],
                                         ones32[0:68, :], et[0:68, j, NT:2 * NT],
                                         start=False, stop=True,
                                         tile_position=(0, band))
                    for j in range(2):
                        h = 2 * sg + j
                        band = 32 * (h % 4)
                        nc.tensor.matmul(avps[band:band + 32, 0:NT],
                                         v_sb[:, 0, h, :], et[:, j, 0:NT],
                                         start=True, stop=False,
                                         tile_position=(0, band))
                        nc.tensor.matmul(avps[band:band + 32, 0:NT],
                                         v_sb[0:68, 1, h, :], et[0:68, j, NT:2 * NT],
                                         start=False, stop=True,
                                         tile_position=(0, band))
                    if sg % 2 == 1:
                        g = sg // 2
                        r_d = apool.tile([128, NT], fp16, tag="rd")
                        with nc.allow_low_precision(reason="softmax recip in fp16"):
                            nc.vector.reciprocal(r_d[:], avps[:, 256:256 + NT])
                        nc.vector.tensor_mul(attn_rs[g][:], avps[:, 0:NT], r_d[:])
                        if proj_hook is not None:
                            proj_hook(g, attn_rs[g])
                return attn_rs

            def proj_emit(w, wi, attn_rs):
                # deferred (w1) proj borrows the attention psum pool: during
                # the next pair's qkv stage it is idle, so its banks don't
                # steal the qpv rotation from under the matmuls.
                for tch, tsz in ((0, 128), (1, 68)):
                    if wi == 0:
                        pp = ps_ms.tile([128, 512], fp32, tag="ms", name="pp")
                    else:
                        pp = ps_av.tile([128, 512], fp32, tag="avps", name="ppd")
                    for bl in range(4):
                        nc.tensor.matmul(pp[0:tsz, 0:CH],
                                         attn_rs[bl][:, tch * 128:tch * 128 + tsz],
                                         pw[:, bl, :], start=(bl == 0), stop=(bl == 3))
                    yt = wpool.tile([128, CH], fp32, tag=f"yt{wi}{tch}")
                    nc.vector.tensor_add(yt[0:tsz, :], pp[0:tsz, 0:CH], b_bcast[0:tsz, :])
                    nc.sync.dma_start(y.ap()[w, tch * 128:tch * 128 + tsz, :], yt[0:tsz, :])

            # ---------------- main loop ----------------
            for pair in range(WPC // 2):
                # prefetch next pair's x before anything else hits SP's queue
                xs_next = load_x(pair + 1) if pair + 1 < WPC // 2 else None
                xr8, x8 = xs_cur

                # q,k feature-major fp8 [128, slot, blk, 392] (fp8 DoubleRow
                # matmuls); softmax 1/sqrt(d) is applied later in the exp.
                qk8 = qk8s[pair % 2]
                q0 = []
                # block order (0,4,...) evacuates exactly the two blocks the
                # first QK sub-groups read (heads 0-3 use qblk 0, kblk 4), so
                # both prefills become legal after just two evacuations
                for mi, mb in enumerate((0, 4, 1, 5, 2, 6, 3, 7)):
                    qpv = ps_ms.tile([128, 512], fp32, tag="ms", name="qpv")
                    for t in range(2):
                        nc.tensor.matmul(qpv[:, 0:2 * NT],
                                         wqk[:, 2 * t:2 * t + 2, mb * 128:(mb + 1) * 128],
                                         x8[:, 2 * t:2 * t + 2, 0:2 * NT],
                                         start=(t == 0), stop=(t == 1), perf_mode=DR)
                    # undo the host fp8 weight pre-scale (x64). Evacs alternate
                    # ACT/DVE: the DR matmuls are so cheap the stage is
                    # evac-paced, so one engine alone would bottleneck it.
                    if mi % 2 == 0:
                        nc.scalar.activation(qk8[:, 0, mb, :], qpv[:, 0:2 * NT],
                                             AF.Copy, scale=1.0 / 64.0)
                    else:
                        nc.vector.tensor_scalar_mul(qk8[:, 0, mb, :], qpv[:, 0:2 * NT],
                                                    1.0 / 64.0)
                    if mi == 2 and pending:
                        pending.pop()()  # prev pair's w1 proj fills this stage
                    if mi == 1:
                        q0.append(emit_qk(qk8, 0, 0))
                        q0.append(emit_qk(qk8, 0, 1))

                # v token-major [128(tok), 2(chunk), H, D] bf16, per window.
                # 3-term fp8 DoubleRow: x8@wv + x8@wvr + xr8@wv, all carrying
                # the x64 weight pre-scale, undone in the evac.
                vs = [None, None]
                for wi in range(2):
                    wo = wi * NT
                    v_sb = wpool.tile([128, 2, H, D], bf16, tag=f"v{wi}")
                    vs[wi] = v_sb
                    for tch, tsz in ((0, 128), (1, 68)):
                        vpv = ps_ms.tile([128, 512], fp32, tag="ms", name="vpv")
                        mi_ = 0
                        for lhs, rhs in ((x8, wv), (x8, wvr), (xr8, wv)):
                            for t in range(2):
                                nc.tensor.matmul(
                                    vpv[0:tsz, 0:CH],
                                    lhs[:, 2 * t:2 * t + 2,
                                        wo + tch * 128: wo + tch * 128 + tsz],
                                    rhs[:, 2 * t:2 * t + 2, :],
                                    start=(mi_ == 0), stop=(mi_ == 5),
                                    perf_mode=DR)
                                mi_ += 1
                        # each evac split into ACT+DVE column halves: the
                        # psum bank frees in half the time and v_sb is ready
                        # before the first AV matmul needs it
                        v_flat = v_sb[0:tsz, tch, :, :].rearrange("p h d -> p (h d)")
                        nc.scalar.activation(v_flat[:, 0:256], vpv[0:tsz, 0:256],
                                             AF.Copy, scale=1.0 / 64.0)
                        nc.vector.tensor_scalar_mul(v_flat[:, 256:CH],
                                                    vpv[0:tsz, 256:CH], 1.0 / 64.0)

                attn_r0 = attn_window(qk8, vs[0], 0, q0)
                q1 = [emit_qk(qk8, 1, 0)]
                proj_emit(2 * pair, 0, attn_r0)
                if pair == WPC // 2 - 1:
                    # last pair: interleave w1's proj into the attention
                    # stream (block g right after group g's normalize) so the
                    # tail is just the final yt+DMA instead of a full proj
                    pp_last = {}

                    def hook(g, ar, w=2 * pair + 1):
                        if g == 0:
                            for tch in range(2):
                                pp_last[tch] = ps_ms.tile([128, 512], fp32,
                                                          tag="ms", name=f"ppl{tch}")
                        for tch, tsz in ((0, 128), (1, 68)):
                            nc.tensor.matmul(pp_last[tch][0:tsz, 0:CH],
                                             ar[:, tch * 128:tch * 128 + tsz],
                                             pw[:, g, :], start=(g == 0), stop=(g == 3))
                        if g == 3:
                            for tch, tsz in ((0, 128), (1, 68)):
                                yt = wpool.tile([128, CH], fp32, tag=f"yt1{tch}")
                                nc.vector.tensor_add(yt[0:tsz, :],
                                                     pp_last[tch][0:tsz, 0:CH],
                                                     b_bcast[0:tsz, :])
                                nc.sync.dma_start(
                                    y.ap()[w, tch * 128:tch * 128 + tsz, :],
                                    yt[0:tsz, :])

                    attn_window(qk8, vs[1], 1, q1, proj_hook=hook)
                else:
                    attn_r1 = attn_window(qk8, vs[1], 1, q1)
                    pending.append(
                        lambda w=2 * pair + 1, a=attn_r1: proj_emit(w, 1, a))

                xs_cur = xs_next

            while pending:
                pending.pop()()

    nc.compile()
    return nc


def _prep_ebt(rel_pos_index, rel_bias_table):
    # ebt[p, h, khi*196 + q] = exp(table[idx[q, p + 128*khi], h]) (1.0 where k pad)
    idx = np.asarray(rel_pos_index).astype(np.int64)
    table = np.asarray(rel_bias_table, dtype=np.float32)
    g = table[idx]                      # [q, k, H]
    out = np.zeros((256, H, NT), dtype=np.float32)
    out[:NT] = g.transpose(1, 2, 0)     # [k, H, q]
    out = np.exp(out)
    return np.ascontiguousarray(
        out.reshape(2, 128, H, NT).transpose(1, 2, 0, 3).reshape(128, H, 2 * NT))


def kernel(x, qkv_w, rel_bias_table, proj_w, proj_b, rel_pos_index):
    import ml_dtypes
    from concourse.bass_utils import run_bass_kernel_spmd

    if "nc" not in _CACHE:
        _CACHE["nc"] = _build()
    nc = _CACHE["nc"]

    bf16 = ml_dtypes.bfloat16
    f8 = ml_dtypes.float8_e4m3
    # host pre-transpose: [b/2, ki, ko, wi*196+q] = x[w, q, 128*ko+ki]
    xf = np.asarray(x, dtype=np.float32).reshape(B // 2, 2, NT, 4, 128)
    xf = np.ascontiguousarray(xf.transpose(0, 4, 3, 1, 2).reshape(B // 2, 128, 4, 2 * NT))
    x8 = xf.astype(f8)
    xr8 = (xf - x8.astype(np.float32)).astype(f8)
    qkv_f = np.asarray(qkv_w, dtype=np.float32)
    # x64 pre-scale lifts the tiny weights out of fp8 subnormal range;
    # undone (with the softmax scale for q) in the on-device evacuation.
    wqk = np.ascontiguousarray(qkv_f[:, :2 * CH] * 64.0).astype(f8)
    wv64 = np.ascontiguousarray(qkv_f[:, 2 * CH:] * 64.0)
    wv = wv64.astype(f8)
    wvr = (wv64 - wv.astype(np.float32)).astype(f8)
    ebt = _prep_ebt(rel_pos_index, rel_bias_table).astype(bf16)
    pw = np.ascontiguousarray(np.asarray(proj_w, dtype=np.float32)).astype(bf16)
    pb = np.ascontiguousarray(np.asarray(proj_b), dtype=np.float32).reshape(1, CH)
    pb16 = pb.astype(bf16)

    hw = WPC // 2
    in_maps = []
    for c in range(NCORES):
        in_maps.append({
            "x8": x8[c * hw:(c + 1) * hw],
            "xr8": xr8[c * hw:(c + 1) * hw],
            "wqk": wqk,
            "wv": wv,
            "wvr": wvr,
            "ebt": ebt,
            "proj_w": pw,
            "proj_b": pb,
            "pb16": pb16,
        })
    res = run_bass_kernel_spmd(nc, in_maps, core_ids=list(range(NCORES)))
    out = np.concatenate([r["y"] for r in res.results], axis=0)
    return out.astype(np.float32)


if __name__ == "__main__":
    pass



# revision 4
# speedup vs baseline: 1.0125x; 1.0125x over previous
"""Swin-style window attention kernel for 8 TRN2 NeuronCores (SPMD, batch-sharded).

Layout strategy per core (16 windows, processed in pairs):
  - host prep: x pre-transposed to [pair, ki, ko, tok] in fp8e4m3 plus an
    fp8 residual copy xr8 = fp8(x - fp8(x)); q,k,v weights x64-scaled to
    fp8 (wv also gets an fp8 residual wvr); proj weights bf16; exp(rel-pos
    bias) precomputed as a bf16 table.
  - q,k projection: fp8 DoubleRow matmuls (2 k-tiles/instr, 0.5 cyc/row),
    evacuated fp8 into persistent ping-pong tiles whose second slot is
    all-zero - the zero slot is the mate k-tile that lets the 32-deep
    QK^T contraction also run in DoubleRow mode at 0.5 cyc/row.
  - v projection token-major [tok,(h,d)] via 3-term fp8 DoubleRow
    (x8@wv + x8@wvr + xr8@wv, one shared x64-scaled psum): ~bf16 accuracy
    at fp8-DR cost. x8/xr8 sbuf tiles are 400-padded (DR Ldweights needs
    Ko stride % 16 == 0).
  - attention in 2-head sub-groups, double-buffered QK psum: QK^T
    transposed (attnT [k,196]) fp8-DR 32-row-packed matmuls; exp on ACT
    with the softmax 1/sqrt(d) as activation pre-scale; *exp(bias) on DVE
    (bf16 2x); AV dense + replicated denominators via ones-matmuls into a
    4-head psum bank; reciprocal + normalize on DVE into per-group
    attn_r tiles (so proj block g depends only on group g).
  - pipeline: w0's first QK sub-groups issued mid-qkv; QK(sg+2) staggered
    ahead of AV(sg); w0 proj inline after w1's first QK; w1 proj deferred
    into the next pair's qkv stage on the idle attention-psum pool. Last
    pair: w1's proj interleaved per-group into the attention stream.
  - proj bf16, bias added during psum evacuation (DVE); y stores + x
    prefetch on the SP queue, weights on ACT hwdge, exp(bias) table in
    4-head chunks on gpsimd swdge.
"""

import numpy as np

B, NT, CH = 128, 196, 512
H, D = 16, 32
NCORES = 8
WPC = B // NCORES  # windows per core

_CACHE = {}


def _build():
    import concourse.bass as bass
    import concourse.mybir as mybir
    import concourse.tile as tile
    from concourse import bacc

    fp32 = mybir.dt.float32
    bf16 = mybir.dt.bfloat16
    fp16 = mybir.dt.float16
    f8 = mybir.dt.float8e4
    DR = mybir.MatmulPerfMode.DoubleRow
    AF = mybir.ActivationFunctionType

    nc = bacc.Bacc("TRN2", target_bir_lowering=False, debug=False, num_devices=NCORES)

    # x pre-transposed on host: [pair, ki, ko, wi*196+q] = x[w, q, 128*ko+ki]
    # fp8 copy feeds q,k AND v (v precision recovered via the fp8 residual
    # copy xr8 and residual weights wvr8: x@wv ~ x8@wv8 + x8@wvr8 + xr8@wv8,
    # all three terms x64-scaled so they share one psum).
    x8d = nc.dram_tensor("x8", [WPC // 2, 128, 4, 2 * NT], f8, kind="ExternalInput")
    xr8d = nc.dram_tensor("xr8", [WPC // 2, 128, 4, 2 * NT], f8, kind="ExternalInput")
    wqk_d = nc.dram_tensor("wqk", [CH, 2 * CH], f8, kind="ExternalInput")
    wv_d = nc.dram_tensor("wv", [CH, CH], f8, kind="ExternalInput")
    wvr_d = nc.dram_tensor("wvr", [CH, CH], f8, kind="ExternalInput")
    ebt_d = nc.dram_tensor("ebt", [128, H, 2 * NT], bf16, kind="ExternalInput")
    proj_w = nc.dram_tensor("proj_w", [CH, CH], bf16, kind="ExternalInput")
    proj_b = nc.dram_tensor("proj_b", [1, CH], fp32, kind="ExternalInput")
    pb16_d = nc.dram_tensor("pb16", [1, CH], bf16, kind="ExternalInput")
    y = nc.dram_tensor("y", [WPC, NT, CH], bf16, kind="ExternalOutput")

    with tile.TileContext(nc) as tc:
        with (
            tc.tile_pool(name="const", bufs=1) as cpool,
            tc.tile_pool(name="work", bufs=3) as wpool,
            tc.tile_pool(name="attn", bufs=8) as apool,
            tc.tile_pool(name="ps_qk", bufs=2, space="PSUM") as ps_qk,
            tc.tile_pool(name="ps_av", bufs=2, space="PSUM") as ps_av,
            tc.tile_pool(name="ps_ms", bufs=2, space="PSUM") as ps_ms,
        ):
            # ---------------- one-time setup ----------------
            # persistent fp8 q,k feature tiles (ping-pong across pairs).
            # Slot a=1 stays all-zero: it is the second k-tile of every
            # DoubleRow QK^T matmul (the PE streams 2 tiles/cycle in DR mode,
            # so a 32-deep contraction runs at 0.5 cyc/row with a zero mate).
            # First tile zeroed on idle DVE right away (needed ~4us in);
            # second on Pool (not needed until pair 1).
            qk8s = []
            for pp_ in range(2):
                qk8t = cpool.tile([128, 2, 8, 2 * NT], f8, tag=f"qk8_{pp_}",
                                  name=f"qk8_{pp_}")
                nc.gpsimd.memset(qk8t[:, 1, :, :], 0.0)
                qk8s.append(qk8t)

            # x loads own the SP queue; weights go via ACT hwdge (in
            # first-needed order); small/late tensors via gpsimd swdge.
            def load_x(pair):
                # x8 split at the DoubleRow kt boundary: the first qkv
                # matmul only needs chunks 0-1, so it can start ~1us sooner.
                # xr8 (residual) is only needed at the v stage, so it loads
                # after both x8 chunks.
                # inner padded 392->400: DoubleRow Ldweights needs the Ko
                # slot stride to be a multiple of 16
                x8 = wpool.tile([128, 4, 400], f8, tag=f"x8{pair % 2}")
                xr8 = wpool.tile([128, 4, 400], f8, tag=f"xr8{pair % 2}")
                nc.sync.dma_start(x8[:, 0:2, 0:2 * NT], x8d.ap()[pair, :, 0:2, :])
                nc.sync.dma_start(x8[:, 2:4, 0:2 * NT], x8d.ap()[pair, :, 2:4, :])
                nc.sync.dma_start(xr8[:, 0:2, 0:2 * NT], xr8d.ap()[pair, :, 0:2, :])
                nc.sync.dma_start(xr8[:, 2:4, 0:2 * NT], xr8d.ap()[pair, :, 2:4, :])
                return xr8, x8

            xs_cur = load_x(0)

            wqk = cpool.tile([128, 4, 2 * CH], f8, tag="wqk")
            wqk_r = wqk_d.ap().rearrange("(ko ki) m -> ki ko m", ki=128)
            nc.scalar.dma_start(wqk[:, 0:2, :], wqk_r[:, 0:2, :])
            nc.scalar.dma_start(wqk[:, 2:4, :], wqk_r[:, 2:4, :])
            wv = cpool.tile([128, 4, CH], f8, tag="wv")
            nc.scalar.dma_start(wv[:], wv_d.ap().rearrange("(ko ki) m -> ki ko m", ki=128))
            wvr = cpool.tile([128, 4, CH], f8, tag="wvr")
            nc.scalar.dma_start(wvr[:], wvr_d.ap().rearrange("(ko ki) m -> ki ko m", ki=128))
            # ebt in 4-head chunks so the big table transfer never delays the
            # startup-critical x/weight DMAs; chunk g unblocks groups in order
            ebt = cpool.tile([128, H, 2 * NT], bf16, tag="ebt")
            for g4 in range(4):
                nc.gpsimd.dma_start(ebt[:, 4 * g4:4 * g4 + 4, :],
                                    ebt_d.ap()[:, 4 * g4:4 * g4 + 4, :])
            pw = cpool.tile([128, 4, CH], bf16, tag="pw")
            nc.scalar.dma_start(pw[:], proj_w.ap().rearrange("(ko ki) m -> ki ko m", ki=128))

            b_row = cpool.tile([1, CH], fp32, tag="brow")
            nc.gpsimd.dma_start(b_row[:], proj_b.ap())
            b_bcast = cpool.tile([128, CH], fp32, tag="bb")
            nc.gpsimd.partition_broadcast(b_bcast[:], b_row[:], channels=128)

            ones32 = cpool.tile([128, 32], bf16, tag="ones")
            nc.gpsimd.memset(ones32[:], 1.0)
            # tail-only: bias row (bf16) + ones row for the last pair's
            # rank-1 bias matmul
            b16 = cpool.tile([1, CH], bf16, tag="b16")
            nc.gpsimd.dma_start(b16[:], pb16_d.ap())
            ones_row = cpool.tile([1, 128], bf16, tag="onesr")
            nc.gpsimd.memset(ones_row[:], 1.0)

            pending = []  # deferred proj emission (fills next pair's qkv stage)

            def emit_qk(qk8, wi, sg):
                wo = wi * NT
                qkps = ps_qk.tile([128, 2, 512], fp32, tag="qkps")
                for j in range(2):
                    h = 2 * sg + j
                    hb = 32 * (h % 4)
                    qblk, kblk = h // 4, 4 + h // 4
                    rhs_q = qk8[hb:hb + 32, :, qblk, wo:wo + NT]
                    nc.tensor.matmul(qkps[:, j, 0:NT],
                                     qk8[hb:hb + 32, :, kblk, wo:wo + 128],
                                     rhs_q, start=True, stop=True,
                                     tile_position=(hb, 0), perf_mode=DR)
                    if wi == 0:
                        nc.tensor.matmul(qkps[:, j, NT:2 * NT],
                                         qk8[hb:hb + 32, :, kblk, 128:256],
                                         rhs_q, start=True, stop=True,
                                         tile_position=(hb, 0), perf_mode=DR)
                    else:
                        nc.tensor.matmul(qkps[0:68, j, NT:2 * NT],
                                         qk8[hb:hb + 32, :, kblk, wo + 128:wo + NT],
                                         rhs_q, start=True, stop=True,
                                         tile_position=(hb, 0), perf_mode=DR)
                return qkps

            def attn_window(qk8, v_sb, wi, qlist, proj_hook=None):
                # qlist: already-emitted qkps tiles (pipeline pre-fill)
                P = len(qlist)
                # one tile per 4-head group so proj block bl only depends on
                # its own group's normalize (not the last one)
                attn_rs = [apool.tile([128, NT], bf16, tag=f"attn_r{g}",
                                      name=f"attn_r{g}")
                           for g in range(4)]
                avps = None
                for sg in range(8):
                    qkps = qlist[sg]
                    # exp (ACT) then *exp(bias); the softmax 1/sqrt(d) rides
                    # along as the activation pre-scale. The bias multiply
                    # alternates DVE/Pool so neither elementwise engine
                    # serializes the attention stream.
                    esb = apool.tile([128, 2, 2 * NT], bf16, tag="esb")
                    nc.scalar.activation(esb[:], qkps[:, :, 0:2 * NT], AF.Exp,
                                         scale=0.17677669529663687)
                    et = apool.tile([128, 2, 2 * NT], bf16, tag="et")
                    nc.vector.tensor_mul(et[:], esb[:], ebt[:, 2 * sg:2 * sg + 2, :])
                    if sg + P < 8:
                        qlist.append(emit_qk(qk8, wi, sg + P))
                    # AV + replicated denominators into 4-head psum bank
                    if sg % 2 == 0:
                        avps = ps_av.tile([128, 512], fp32, tag="avps")
                    # ones (denominator) matmuls FIRST: the reciprocal can
                    # then overlap the AV matmuls instead of serializing
                    # after them on the window-end critical path
                    for j in range(2):
                        h = 2 * sg + j
                        band = 32 * (h % 4)
                        nc.tensor.matmul(avps[band:band + 32, 256:256 + NT],
                                         ones32[:], et[:, j, 0:NT],
                                         start=True, stop=False,
                                         tile_position=(0, band))
                        nc.tensor.matmul(avps[band:band + 32, 256:256 + NT],
                                         ones32[0:68, :], et[0:68, j, NT:2 * NT],
                                         start=False, stop=True,
                                         tile_position=(0, band))
                    for j in range(2):
                        h = 2 * sg + j
                        band = 32 * (h % 4)
                        nc.tensor.matmul(avps[band:band + 32, 0:NT],
                                         v_sb[:, 0, h, :], et[:, j, 0:NT],
                                         start=True, stop=False,
                                         tile_position=(0, band))
                        nc.tensor.matmul(avps[band:band + 32, 0:NT],
                                         v_sb[0:68, 1, h, :], et[0:68, j, NT:2 * NT],
                                         start=False, stop=True,
                                         tile_position=(0, band))
                    if sg % 2 == 1:
                        g = sg // 2
                        r_d = apool.tile([128, NT], fp16, tag="rd")
                        with nc.allow_low_precision(reason="softmax recip in fp16"):
                            nc.vector.reciprocal(r_d[:], avps[:, 256:256 + NT])
                        nc.vector.tensor_mul(attn_rs[g][:], avps[:, 0:NT], r_d[:])
                        if proj_hook is not None:
                            proj_hook(g, attn_rs[g])
                return attn_rs

            def proj_emit(w, wi, attn_rs):
                # deferred (w1) proj borrows the attention psum pool: during
                # the next pair's qkv stage it is idle, so its banks don't
                # steal the qpv rotation from under the matmuls.
                for tch, tsz in ((0, 128), (1, 68)):
                    if wi == 0:
                        pp = ps_ms.tile([128, 512], fp32, tag="ms", name="pp")
                    else:
                        pp = ps_av.tile([128, 512], fp32, tag="avps", name="ppd")
                    for bl in range(4):
                        nc.tensor.matmul(pp[0:tsz, 0:CH],
                                         attn_rs[bl][:, tch * 128:tch * 128 + tsz],
                                         pw[:, bl, :], start=(bl == 0), stop=(bl == 3))
                    yt = wpool.tile([128, CH], bf16, tag=f"yt{wi}{tch}")
                    with nc.allow_low_precision(reason="y rounds to bf16 at the store"):
                        nc.vector.tensor_add(yt[0:tsz, :], pp[0:tsz, 0:CH], b_bcast[0:tsz, :])
                    nc.sync.dma_start(y.ap()[w, tch * 128:tch * 128 + tsz, :], yt[0:tsz, :])

            # ---------------- main loop ----------------
            for pair in range(WPC // 2):
                # prefetch next pair's x before anything else hits SP's queue
                xs_next = load_x(pair + 1) if pair + 1 < WPC // 2 else None
                xr8, x8 = xs_cur

                # q,k feature-major fp8 [128, slot, blk, 392] (fp8 DoubleRow
                # matmuls); softmax 1/sqrt(d) is applied later in the exp.
                qk8 = qk8s[pair % 2]
                q0 = []
                # block order (0,4,...) evacuates exactly the two blocks the
                # first QK sub-groups read (heads 0-3 use qblk 0, kblk 4), so
                # both prefills become legal after just two evacuations
                for mi, mb in enumerate((0, 4, 1, 5, 2, 6, 3, 7)):
                    qpv = ps_ms.tile([128, 512], fp32, tag="ms", name="qpv")
                    for t in range(2):
                        nc.tensor.matmul(qpv[:, 0:2 * NT],
                                         wqk[:, 2 * t:2 * t + 2, mb * 128:(mb + 1) * 128],
                                         x8[:, 2 * t:2 * t + 2, 0:2 * NT],
                                         start=(t == 0), stop=(t == 1), perf_mode=DR)
                    # undo the host fp8 weight pre-scale (x64). Evacs alternate
                    # ACT/DVE: the DR matmuls are so cheap the stage is
                    # evac-paced, so one engine alone would bottleneck it.
                    if mi % 2 == 0:
                        nc.scalar.activation(qk8[:, 0, mb, :], qpv[:, 0:2 * NT],
                                             AF.Copy, scale=1.0 / 64.0)
                    else:
                        nc.vector.tensor_scalar_mul(qk8[:, 0, mb, :], qpv[:, 0:2 * NT],
                                                    1.0 / 64.0)
                    if mi == 2 and pending:
                        pending.pop()()  # prev pair's w1 proj fills this stage
                    if mi == 1:
                        q0.append(emit_qk(qk8, 0, 0))
                        q0.append(emit_qk(qk8, 0, 1))

                # v token-major [128(tok), 2(chunk), H, D] bf16, per window.
                # 3-term fp8 DoubleRow: x8@wv + x8@wvr + xr8@wv, all carrying
                # the x64 weight pre-scale, undone in the evac.
                vs = [None, None]
                for wi in range(2):
                    wo = wi * NT
                    v_sb = wpool.tile([128, 2, H, D], bf16, tag=f"v{wi}")
                    vs[wi] = v_sb
                    for tch, tsz in ((0, 128), (1, 68)):
                        vpv = ps_ms.tile([128, 512], fp32, tag="ms", name="vpv")
                        mi_ = 0
                        for lhs, rhs in ((x8, wv), (x8, wvr), (xr8, wv)):
                            for t in range(2):
                                nc.tensor.matmul(
                                    vpv[0:tsz, 0:CH],
                                    lhs[:, 2 * t:2 * t + 2,
                                        wo + tch * 128: wo + tch * 128 + tsz],
                                    rhs[:, 2 * t:2 * t + 2, :],
                                    start=(mi_ == 0), stop=(mi_ == 5),
                                    perf_mode=DR)
                                mi_ += 1
                        # each evac split into ACT+DVE column halves: the
                        # psum bank frees in half the time and v_sb is ready
                        # before the first AV matmul needs it
                        v_flat = v_sb[0:tsz, tch, :, :].rearrange("p h d -> p (h d)")
                        nc.scalar.activation(v_flat[:, 0:256], vpv[0:tsz, 0:256],
                                             AF.Copy, scale=1.0 / 64.0)
                        nc.vector.tensor_scalar_mul(v_flat[:, 256:CH],
                                                    vpv[0:tsz, 256:CH], 1.0 / 64.0)

                attn_r0 = attn_window(qk8, vs[0], 0, q0)
                q1 = [emit_qk(qk8, 1, 0)]
                proj_emit(2 * pair, 0, attn_r0)
                if pair == WPC // 2 - 1:
                    # last pair: interleave w1's proj into the attention
                    # stream (block g right after group g's normalize) so the
                    # tail is just the final yt+DMA instead of a full proj
                    pp_last = {}

                    def hook(g, ar, w=2 * pair + 1):
                        if g == 0:
                            for tch in range(2):
                                pp_last[tch] = ps_ms.tile([128, 512], fp32,
                                                          tag="ms", name=f"ppl{tch}")
                        for tch, tsz in ((0, 128), (1, 68)):
                            nc.tensor.matmul(pp_last[tch][0:tsz, 0:CH],
                                             ar[:, tch * 128:tch * 128 + tsz],
                                             pw[:, g, :], start=(g == 0), stop=(g == 3))
                        if g == 3:
                            for tch, tsz in ((0, 128), (1, 68)):
                                yt = wpool.tile([128, CH], bf16, tag=f"yt1{tch}")
                                with nc.allow_low_precision(reason="y rounds to bf16 at the store"):
                                    nc.vector.tensor_add(yt[0:tsz, :],
                                                         pp_last[tch][0:tsz, 0:CH],
                                                         b_bcast[0:tsz, :])
                                nc.sync.dma_start(
                                    y.ap()[w, tch * 128:tch * 128 + tsz, :],
                                    yt[0:tsz, :])

                    attn_window(qk8, vs[1], 1, q1, proj_hook=hook)
                else:
                    attn_r1 = attn_window(qk8, vs[1], 1, q1)
                    pending.append(
                        lambda w=2 * pair + 1, a=attn_r1: proj_emit(w, 1, a))

                xs_cur = xs_next

            while pending:
                pending.pop()()

    nc.compile()
    return nc


def _prep_ebt(rel_pos_index, rel_bias_table):
    # ebt[p, h, khi*196 + q] = exp(table[idx[q, p + 128*khi], h]) (1.0 where k pad)
    idx = np.asarray(rel_pos_index).astype(np.int64)
    table = np.asarray(rel_bias_table, dtype=np.float32)
    g = table[idx]                      # [q, k, H]
    out = np.zeros((256, H, NT), dtype=np.float32)
    out[:NT] = g.transpose(1, 2, 0)     # [k, H, q]
    out = np.exp(out)
    return np.ascontiguousarray(
        out.reshape(2, 128, H, NT).transpose(1, 2, 0, 3).reshape(128, H, 2 * NT))


def kernel(x, qkv_w, rel_bias_table, proj_w, proj_b, rel_pos_index):
    import ml_dtypes
    from concourse.bass_utils import run_bass_kernel_spmd

    if "nc" not in _CACHE:
        _CACHE["nc"] = _build()
    nc = _CACHE["nc"]

    bf16 = ml_dtypes.bfloat16
    f8 = ml_dtypes.float8_e4m3
    # host pre-transpose: [b/2, ki, ko, wi*196+q] = x[w, q, 128*ko+ki]
    xf = np.asarray(x, dtype=np.float32).reshape(B // 2, 2, NT, 4, 128)
    xf = np.ascontiguousarray(xf.transpose(0, 4, 3, 1, 2).reshape(B // 2, 128, 4, 2 * NT))
    x8 = xf.astype(f8)
    xr8 = (xf - x8.astype(np.float32)).astype(f8)
    qkv_f = np.asarray(qkv_w, dtype=np.float32)
    # x64 pre-scale lifts the tiny weights out of fp8 subnormal range;
    # undone (with the softmax scale for q) in the on-device evacuation.
    wqk = np.ascontiguousarray(qkv_f[:, :2 * CH] * 64.0).astype(f8)
    wv64 = np.ascontiguousarray(qkv_f[:, 2 * CH:] * 64.0)
    wv = wv64.astype(f8)
    wvr = (wv64 - wv.astype(np.float32)).astype(f8)
    ebt = _prep_ebt(rel_pos_index, rel_bias_table).astype(bf16)
    pw = np.ascontiguousarray(np.asarray(proj_w, dtype=np.float32)).astype(bf16)
    pb = np.ascontiguousarray(np.asarray(proj_b), dtype=np.float32).reshape(1, CH)
    pb16 = pb.astype(bf16)

    hw = WPC // 2
    in_maps = []
    for c in range(NCORES):
        in_maps.append({
            "x8": x8[c * hw:(c + 1) * hw],
            "xr8": xr8[c * hw:(c + 1) * hw],
            "wqk": wqk,
            "wv": wv,
            "wvr": wvr,
            "ebt": ebt,
            "proj_w": pw,
            "proj_b": pb,
            "pb16": pb16,
        })
    res = run_bass_kernel_spmd(nc, in_maps, core_ids=list(range(NCORES)))
    out = np.concatenate([np.asarray(r["y"], dtype=np.float32) for r in res.results],
                         axis=0)
    return out


if __name__ == "__main__":
    pass

